# revision 9
# baseline (speedup 1.0000x reference)
"""Trainium2 Bass kernel for the DActor dense MLP.

Network (per row of `state`):
    h1 = relu(state @ W1 + b1)        # 512 -> 500
    h2 = relu(h1 @ W2 + b2)           # 500 -> 300
    h3 = relu(h2 @ W3 + b3)           # 300 -> 100
    v  = h3 @ W4 + b4                 # 100 -> 64
    t  = tanh(v[:, :63]); s = sigmoid(v[:, 63:])
    possum = sum(relu(t)); denom = possum == 0 ? 1 : possum
    out = concat(where(t > 0, t / denom, t), s)

Strategy: pure data parallel over 8 NeuronCores (8192 rows each).
Activations stay feature-major ([feat, batch]) through L1-L3 so every matmul
uses [fan_in, fan_out] weight tiles as the stationary operand with 512-wide
moving batch chunks. All matmul operands are bf16 (fp32 accumulate in PSUM):
same 1 col/cycle PE rate as float32r but half the DMA/SBUF traffic and
cheaper weight loads. Validated: max abs err ~1.1e-3, l2 rel ~2e-3.

Biases b2/b3/b4 are folded into padded weight rows: the host writes b_n into
a zero-padded row of W_n and plants a 1.0 marker so each layer's activation
vector carries a constant-1 feature (h1[500]=1 via b1 pad, h2[300]=1 via W2
marker, h3[100]=1 via W3 marker). L2/L3 PSUM evictions are then pure relu
and run on DVE/Pool, keeping the Activation engine off the critical path.

L4 is computed batch-major: each 128-row batch block of h3 becomes the
stationary operand with W4 [128k, 64] moving, writing v directly into the
batch-major output PSUM tile. This removes the PE transposes and the
identity matrix of the v1 kernel entirely. The PST epilogue runs on the
PSUM tile: possum = (sum(t) + sum(|t|))/2, out = max(t,0)*recip + min(t,0)
via two fused scalar_tensor_tensor ops (safe: possum >= 1 in practice, and
the all-negative row case yields exactly 0*inf-free results by construction).

Head-latency tricks: W1/x-chunk-0 are DMA'd in interleaved k-tile order with
chunk 0's layer 1 emitted k-outer, so the PE starts after ~256 KB instead of
~3 MB; dummy warmup matmuls during the DMA fill ramp the PE out of its low
p-state before real work arrives.
"""

import os

import numpy as np
import ml_dtypes

import concourse.bass as bass
import concourse.tile as tile
from concourse import bacc, mybir
from concourse.bass_utils import run_bass_kernel_spmd

N_CORES = 8
BATCH = 65536
B = BATCH // N_CORES  # 8192 rows per core
D_IN, H1, H2, H3, D_OUT = 512, 500, 300, 100, 64
NCHUNK = 512  # moving-operand width (1 PSUM bank of fp32)
N_CHUNKS = B // NCHUNK  # 16
GPB = NCHUNK // 128  # 128-row batch blocks per chunk
BLOCKS_PER_BM = 4  # 4 x 64 output cols per [128, 256B] batch-major PSUM tile

K1, K2, K3 = 4, 4, 3  # k-tiles per layer (padded K: 512, 512, 384)
M1, M2 = 4, 3  # m-tiles for L1 (500->512) and L2 (300->384)

F32 = mybir.dt.float32
# bf16 matmuls: 1 col/cycle on the PE (same as float32r) but cheaper weight
# loads and half the DMA bytes. BASS_MM_DTYPE=float32r to fall back.
_MM_NAME = os.environ.get("BASS_MM_DTYPE", "bfloat16")
MM_DT = getattr(mybir.dt, _MM_NAME)
NP_MM = ml_dtypes.bfloat16 if _MM_NAME == "bfloat16" else np.float32
WARMUP = int(os.environ.get("BASS_WARMUP", "52"))


def _emit(tc: tile.TileContext, aps: dict):
    nc = tc.nc
    xT = aps["xT"]
    out = aps["out"]

    consts = tc.alloc_tile_pool(name="consts", bufs=1)
    acts = tc.alloc_tile_pool(name="acts", bufs=3)
    outs = tc.alloc_tile_pool(name="outs", bufs=3)
    scratch = tc.alloc_tile_pool(name="scratch", bufs=2)
    psum_mm = tc.alloc_tile_pool(name="psum_mm", bufs=6, space="PSUM")
    psum_bm = tc.alloc_tile_pool(name="psum_bm", bufs=2, space="PSUM")

    xT_v = xT.rearrange("(k p) b -> p k b", p=128)  # [128, 4, B]
    w1_v = aps["W1"].rearrange("(k p) m -> p k m", p=128)
    # out rows = 1024*j + 128*t + p  ->  [j, p, t, f]
    out_v = out.rearrange("(j t p) f -> j p t f", t=BLOCKS_PER_BM, p=128)

    Relu = mybir.ActivationFunctionType.Relu

    # ---- first-chunk DMAs in k-interleaved order, on the GpSimd sequencer
    # (it comes out of kernel-start init earliest) ------------------------
    w1 = consts.tile([128, K1, 512], MM_DT)
    x0 = acts.tile([128, K1, NCHUNK], MM_DT, tag="x")
    for ki in range(K1):
        nc.gpsimd.dma_start(out=w1[:, ki, :], in_=w1_v[:, ki, :])
        nc.gpsimd.dma_start(out=x0[:, ki, :], in_=xT_v[:, ki, 0:NCHUNK])

    b1 = consts.tile([128, M1], F32)
    nc.gpsimd.dma_start(out=b1, in_=aps["b1"].rearrange("(m p) -> p m", p=128))
    w2 = consts.tile([128, K2, 384], MM_DT)
    nc.gpsimd.dma_start(out=w2, in_=aps["W2"].rearrange("(k p) m -> p k m", p=128))
    w3 = consts.tile([128, K3, 128], MM_DT)
    nc.gpsimd.dma_start(out=w3, in_=aps["W3"].rearrange("(k p) m -> p k m", p=128))
    w4 = consts.tile([128, D_OUT], MM_DT)
    nc.gpsimd.dma_start(out=w4, in_=aps["W4"])

    wtmp = consts.tile([128, 64], MM_DT)
    nc.gpsimd.memset(wtmp, 0.0)

    # ---- PE p-state warmup while the first DMAs land --------------------
    if WARMUP:
        wps = psum_mm.tile([128, NCHUNK], F32, tag="ps")
        for _ in range(WARMUP):
            nc.tensor.matmul(wps[0:64, 0:64], wtmp[:, 0:64], wtmp[:, 0:64],
                             start=True, stop=True)

    bm = None
    pending_l4 = None  # (chunk, h3 tile) whose L4 matmuls are deferred

    def emit_l4():
        # L4 for the previous chunk, emitted after the next chunk's L1
        # matmuls so the PE never waits on the Pool-produced h3.
        nonlocal pending_l4, bm
        if pending_l4 is None:
            return
        pc, ph3 = pending_l4
        pending_l4 = None
        for bb in range(GPB):
            g = pc * GPB + bb
            t = g % BLOCKS_PER_BM
            if t == 0:
                bm = psum_bm.tile([128, BLOCKS_PER_BM, D_OUT], F32, tag="bm")
            # batch-major L4: stationary = h3 batch block, moving = W4
            nc.tensor.matmul(bm[:, t, :], ph3[:, bb * 128:(bb + 1) * 128], w4,
                             start=True, stop=True)
            if t == BLOCKS_PER_BM - 1:
                _pst_store(nc, scratch, outs, bm, out_v, g // BLOCKS_PER_BM)

    for c in range(N_CHUNKS):
        # ---- layer 1: [512 -> 500(pad 512)] -----------------------------
        h1 = acts.tile([128, K2, NCHUNK], MM_DT, tag="h1")
        if c == 0:
            # k-outer so the PE starts after w1/x k-tile 0 arrives instead
            # of the full weight+chunk load.
            ps_l1 = [psum_mm.tile([128, NCHUNK], F32, tag="ps", name=f"ps_l1_{mi}")
                     for mi in range(M1)]
            for ki in range(K1):
                for mi in range(M1):
                    nc.tensor.matmul(
                        ps_l1[mi], w1[:, ki, mi * 128:(mi + 1) * 128],
                        x0[:, ki, :], start=(ki == 0), stop=(ki == K1 - 1))
            for mi in range(M1):
                nc.scalar.activation(out=h1[:, mi, :], in_=ps_l1[mi],
                                     func=Relu, bias=b1[:, mi:mi + 1])
            x_sb = x0
        else:
            cs = slice(c * NCHUNK, (c + 1) * NCHUNK)
            x_sb = acts.tile([128, K1, NCHUNK], MM_DT, tag="x")
            for ki in range(K1):
                nc.sync.dma_start(out=x_sb[:, ki, :], in_=xT_v[:, ki, cs])
            for mi in range(M1):
                ps = psum_mm.tile([128, NCHUNK], F32, tag="ps")
                msl = slice(mi * 128, (mi + 1) * 128)
                for ki in range(K1):
                    nc.tensor.matmul(ps, w1[:, ki, msl], x_sb[:, ki, :],
                                     start=(ki == 0), stop=(ki == K1 - 1))
                nc.scalar.activation(out=h1[:, mi, :], in_=ps, func=Relu,
                                     bias=b1[:, mi:mi + 1])
        emit_l4()

        # ---- layer 2: [500 -> 300(pad 384)], bias folded into W2 --------
        h2 = acts.tile([128, K3, NCHUNK], MM_DT, tag="h2")
        for mi in range(M2):
            ps = psum_mm.tile([128, NCHUNK], F32, tag="ps")
            msl = slice(mi * 128, (mi + 1) * 128)
            for ki in range(K2):
                nc.tensor.matmul(ps, w2[:, ki, msl], h1[:, ki, :],
                                 start=(ki == 0), stop=(ki == K2 - 1))
            nc.vector.tensor_scalar_max(h2[:, mi, :], ps, 0.0)

        # ---- layer 3: [300 -> 100(pad 128)], bias folded into W3 --------
        h3 = acts.tile([128, NCHUNK], MM_DT, tag="h3")
        ps = psum_mm.tile([128, NCHUNK], F32, tag="ps")
        for ki in range(K3):
            nc.tensor.matmul(ps, w3[:, ki, :], h2[:, ki, :],
                             start=(ki == 0), stop=(ki == K3 - 1))
        nc.scalar.activation(out=h3, in_=ps, func=Relu)

        pending_l4 = (c, h3)

    emit_l4()

    for pool in (psum_bm, psum_mm, scratch, outs, acts, consts):
        pool.release()


def _pst_store(nc, scratch, outs, bm, out_v, j):
    """PST epilogue on one batch-major [128, 8, 64] PSUM tile + store.

    `bm` holds v = x@W4 + b4 (bias folded into the matmul). possum is
    computed as (sum(t) + sum(|t|)) / 2; for all-negative rows both sums
    cancel exactly (identical reduction order) giving possum = 0, which the
    1e-38 floor turns into a huge-but-finite recip multiplied by
    max(t,0) = 0, reproducing the reference's denom=1 semantics.
    """
    G = BLOCKS_PER_BM
    Tanh = mybir.ActivationFunctionType.Tanh
    Sigm = mybir.ActivationFunctionType.Sigmoid
    Op = mybir.AluOpType

    o_sb = outs.tile([128, G, D_OUT], F32, tag="o")
    tb = scratch.tile([128, G, 63], F32, tag="tb")
    nc.scalar.activation(out=tb, in_=bm[:, :, 0:63], func=Tanh)
    nc.scalar.activation(out=o_sb[:, :, 63:64], in_=bm[:, :, 63:64], func=Sigm)

    s1 = scratch.tile([128, G], F32, tag="s1")
    nc.vector.reduce_sum(out=s1, in_=tb, axis=mybir.AxisListType.X)
    sa = scratch.tile([128, G], F32, tag="sa")
    nc.vector.tensor_reduce(out=sa, in_=tb, axis=mybir.AxisListType.X,
                            op=Op.add, apply_absolute_value=True)
    p = scratch.tile([128, G], F32, tag="p")
    nc.vector.tensor_tensor(out=p, in0=s1, in1=sa, op=Op.add)
    pg = scratch.tile([128, G], F32, tag="pg")
    nc.vector.tensor_scalar(out=pg, in0=p, scalar1=0.5, scalar2=1e-38,
                            op0=Op.mult, op1=Op.max)
    r = scratch.tile([128, G], F32, tag="r")
    nc.vector.reciprocal(r, pg)

    rb = r.unsqueeze(2).broadcast_to([128, G, 63])
    z = scratch.tile([128, G, 63], F32, tag="z")
    nc.vector.scalar_tensor_tensor(out=z, in0=tb, scalar=0.0, in1=rb,
                                   op0=Op.max, op1=Op.mult)
    nc.vector.scalar_tensor_tensor(out=o_sb[:, :, 0:63], in0=tb, scalar=0.0,
                                   in1=z, op0=Op.min, op1=Op.add)
    nc.sync.dma_start(out=out_v[j], in_=o_sb)


_PROG_CACHE = {}


def _build():
    if "nc" in _PROG_CACHE:
        return _PROG_CACHE["nc"]
    nc = bacc.Bacc("TRN2", target_bir_lowering=False, debug=False,
                   enable_asserts=False)
    aps = {
        "xT": nc.dram_tensor("xT", [D_IN, B], MM_DT, kind="ExternalInput").ap(),
        "W1": nc.dram_tensor("W1", [D_IN, 512], MM_DT, kind="ExternalInput").ap(),
        "b1": nc.dram_tensor("b1", [512], F32, kind="ExternalInput").ap(),
        "W2": nc.dram_tensor("W2", [512, 384], MM_DT, kind="ExternalInput").ap(),
        "W3": nc.dram_tensor("W3", [384, 128], MM_DT, kind="ExternalInput").ap(),
        "W4": nc.dram_tensor("W4", [128, D_OUT], MM_DT, kind="ExternalInput").ap(),
        "out": nc.dram_tensor("out", [B, D_OUT], F32, kind="ExternalOutput").ap(),
    }
    with tile.TileContext(nc) as tc:
        _emit(tc, aps)
    nc.compile()
    _PROG_CACHE["nc"] = nc
    return nc


def kernel(state, W1, b1, W2, b2, W3, b3, W4, b4, _trace=False):
    nc = _build()

    # Host-side padding: biases b2/b3/b4 are folded into a padded weight row,
    # with 1.0 markers chaining a constant-1 feature through the layers
    # (h1[500] via the b1 pad, h2[300] via W2[500,300], h3[100] via
    # W3[300,100]).
    W1p = np.zeros((512, 512), np.float32)
    W1p[:, :H1] = np.asarray(W1, np.float32)
    b1p = np.zeros((512,), np.float32)
    b1p[:H1] = np.asarray(b1, np.float32)
    b1p[H1] = 1.0
    W2p = np.zeros((512, 384), np.float32)
    W2p[:H1, :H2] = np.asarray(W2, np.float32)
    W2p[H1, :H2] = np.asarray(b2, np.float32)
    W2p[H1, H2] = 1.0
    W3p = np.zeros((384, 128), np.float32)
    W3p[:H2, :H3] = np.asarray(W3, np.float32)
    W3p[H2, :H3] = np.asarray(b3, np.float32)
    W3p[H2, H3] = 1.0
    W4p = np.zeros((128, D_OUT), np.float32)
    W4p[:H3, :] = np.asarray(W4, np.float32)
    W4p[H3, :] = np.asarray(b4, np.float32)

    weights = {
        "W1": W1p.astype(NP_MM), "b1": b1p, "W2": W2p.astype(NP_MM),
        "W3": W3p.astype(NP_MM), "W4": W4p.astype(NP_MM),
    }
    xT_all = np.asarray(state, np.float32).T.astype(NP_MM)  # [512, 65536]
    in_maps = []
    for i in range(N_CORES):
        in_maps.append(
            {"xT": np.ascontiguousarray(xT_all[:, i * B:(i + 1) * B]), **weights})

    res = run_bass_kernel_spmd(nc, in_maps, core_ids=list(range(N_CORES)),
                               trace=_trace)
    full = np.concatenate([res.results[i]["out"] for i in range(N_CORES)], axis=0)
    if _trace:
        kernel.last_results = res
    return full


# revision 11
# speedup vs baseline: 1.0140x; 1.0140x over previous
"""Trainium2 Bass kernel for the DActor dense MLP.

Network (per row of `state`):
    h1 = relu(state @ W1 + b1)        # 512 -> 500
    h2 = relu(h1 @ W2 + b2)           # 500 -> 300
    h3 = relu(h2 @ W3 + b3)           # 300 -> 100
    v  = h3 @ W4 + b4                 # 100 -> 64
    t  = tanh(v[:, :63]); s = sigmoid(v[:, 63:])
    possum = sum(relu(t)); denom = possum == 0 ? 1 : possum
    out = concat(where(t > 0, t / denom, t), s)

Strategy: pure data parallel over 8 NeuronCores (8192 rows each).
Activations stay feature-major ([feat, batch]) through L1-L3 so every matmul
uses [fan_in, fan_out] weight tiles as the stationary operand with 512-wide
moving batch chunks. All matmul operands are bf16 (fp32 accumulate in PSUM):
same 1 col/cycle PE rate as float32r but half the DMA/SBUF traffic and
cheaper weight loads. Validated: max abs err ~1.1e-3, l2 rel ~2e-3.

Biases b2/b3/b4 are folded into padded weight rows: the host writes b_n into
a zero-padded row of W_n and plants a 1.0 marker so each layer's activation
vector carries a constant-1 feature (h1[500]=1 via b1 pad, h2[300]=1 via W2
marker, h3[100]=1 via W3 marker). L2/L3 PSUM evictions are then pure relu
and run on DVE/Pool, keeping the Activation engine off the critical path.

L4 is computed batch-major: each 128-row batch block of h3 becomes the
stationary operand with W4 [128k, 64] moving, writing v directly into the
batch-major output PSUM tile. This removes the PE transposes and the
identity matrix of the v1 kernel entirely. The PST epilogue runs on the
PSUM tile: possum = (sum(t) + sum(|t|))/2, out = max(t,0)*recip + min(t,0)
via two fused scalar_tensor_tensor ops (safe: possum >= 1 in practice, and
the all-negative row case yields exactly 0*inf-free results by construction).

Head-latency tricks: W1/x-chunk-0 are DMA'd in interleaved k-tile order with
chunk 0's layer 1 emitted k-outer, so the PE starts after ~256 KB instead of
~3 MB; dummy warmup matmuls during the DMA fill ramp the PE out of its low
p-state before real work arrives.
"""

import os

import numpy as np
import ml_dtypes

import concourse.bass as bass
import concourse.tile as tile
from concourse import bacc, mybir
from concourse.bass_utils import run_bass_kernel_spmd

N_CORES = 8
BATCH = 65536
B = BATCH // N_CORES  # 8192 rows per core
D_IN, H1, H2, H3, D_OUT = 512, 500, 300, 100, 64
NCHUNK = 512  # moving-operand width (1 PSUM bank of fp32)
N_CHUNKS = B // NCHUNK  # 16
GPB = NCHUNK // 128  # 128-row batch blocks per chunk
BLOCKS_PER_BM = 4  # 4 x 64 output cols per [128, 256B] batch-major PSUM tile

K1, K2, K3 = 4, 4, 3  # k-tiles per layer (padded K: 512, 512, 384)
M1, M2 = 4, 3  # m-tiles for L1 (500->512) and L2 (300->384)

F32 = mybir.dt.float32
# bf16 matmuls: 1 col/cycle on the PE (same as float32r) but cheaper weight
# loads and half the DMA bytes. BASS_MM_DTYPE=float32r to fall back.
_MM_NAME = os.environ.get("BASS_MM_DTYPE", "bfloat16")
MM_DT = getattr(mybir.dt, _MM_NAME)
NP_MM = ml_dtypes.bfloat16 if _MM_NAME == "bfloat16" else np.float32
WARMUP = int(os.environ.get("BASS_WARMUP", "52"))


def _emit(tc: tile.TileContext, aps: dict):
    nc = tc.nc
    xT = aps["xT"]
    out = aps["out"]

    consts = tc.alloc_tile_pool(name="consts", bufs=1)
    acts = tc.alloc_tile_pool(name="acts", bufs=3)
    outs = tc.alloc_tile_pool(name="outs", bufs=3)
    scratch = tc.alloc_tile_pool(name="scratch", bufs=2)
    psum_mm = tc.alloc_tile_pool(name="psum_mm", bufs=6, space="PSUM")
    psum_bm = tc.alloc_tile_pool(name="psum_bm", bufs=2, space="PSUM")

    xT_v = xT.rearrange("(k p) b -> p k b", p=128)  # [128, 4, B]
    w1_v = aps["W1"].rearrange("(k p) m -> p k m", p=128)
    # out rows = 1024*j + 128*t + p  ->  [j, p, t, f]
    out_v = out.rearrange("(j t p) f -> j p t f", t=BLOCKS_PER_BM, p=128)

    Relu = mybir.ActivationFunctionType.Relu

    # ---- head: GpSimd's sequencer wakes ~2.5us before Sync's, but each of
    # its dma_starts costs a ~900ns DIRECT2D descriptor-gen on the engine.
    # So GpSimd gets only the warmup memset + the two k-tile-0 DMAs the
    # first matmul group needs; everything else goes through Sync's cheap
    # hardware queue triggers. ---------------------------------------------
    w1 = consts.tile([128, K1, 512], MM_DT)
    x0 = acts.tile([128, K1, NCHUNK], MM_DT, tag="x")
    wtmp = consts.tile([128, 64], MM_DT)
    nc.gpsimd.memset(wtmp, 0.0)
    nc.gpsimd.dma_start(out=w1[:, 0, :], in_=w1_v[:, 0, :])
    nc.gpsimd.dma_start(out=x0[:, 0, :], in_=xT_v[:, 0, 0:NCHUNK])
    for ki in range(1, K1):
        nc.sync.dma_start(out=w1[:, ki, :], in_=w1_v[:, ki, :])
        nc.sync.dma_start(out=x0[:, ki, :], in_=xT_v[:, ki, 0:NCHUNK])

    b1 = consts.tile([128, M1], F32)
    nc.gpsimd.dma_start(out=b1, in_=aps["b1"].rearrange("(m p) -> p m", p=128))
    w2 = consts.tile([128, K2, 384], MM_DT)
    nc.gpsimd.dma_start(out=w2, in_=aps["W2"].rearrange("(k p) m -> p k m", p=128))
    w3 = consts.tile([128, K3, 128], MM_DT)
    nc.gpsimd.dma_start(out=w3, in_=aps["W3"].rearrange("(k p) m -> p k m", p=128))
    w4 = consts.tile([128, D_OUT], MM_DT)
    nc.gpsimd.dma_start(out=w4, in_=aps["W4"])

    # ---- PE p-state warmup while the first DMAs land --------------------
    if WARMUP:
        wps = psum_mm.tile([128, NCHUNK], F32, tag="ps")
        for _ in range(WARMUP):
            nc.tensor.matmul(wps[0:64, 0:64], wtmp[:, 0:64], wtmp[:, 0:64],
                             start=True, stop=True)

    bm = None
    pending_l4 = None  # (chunk, h3 tile) whose L4 matmuls are deferred

    def emit_l4():
        # L4 for the previous chunk, emitted after the next chunk's L1
        # matmuls so the PE never waits on the Pool-produced h3.
        nonlocal pending_l4, bm
        if pending_l4 is None:
            return
        pc, ph3 = pending_l4
        pending_l4 = None
        for bb in range(GPB):
            g = pc * GPB + bb
            t = g % BLOCKS_PER_BM
            if t == 0:
                bm = psum_bm.tile([128, BLOCKS_PER_BM, D_OUT], F32, tag="bm")
            # batch-major L4: stationary = h3 batch block, moving = W4
            nc.tensor.matmul(bm[:, t, :], ph3[:, bb * 128:(bb + 1) * 128], w4,
                             start=True, stop=True)
            if t == BLOCKS_PER_BM - 1:
                _pst_store(nc, scratch, outs, bm, out_v, g // BLOCKS_PER_BM)

    for c in range(N_CHUNKS):
        # ---- layer 1: [512 -> 500(pad 512)] -----------------------------
        h1 = acts.tile([128, K2, NCHUNK], MM_DT, tag="h1")
        if c == 0:
            # k-outer so the PE starts after w1/x k-tile 0 arrives instead
            # of the full weight+chunk load.
            ps_l1 = [psum_mm.tile([128, NCHUNK], F32, tag="ps", name=f"ps_l1_{mi}")
                     for mi in range(M1)]
            for ki in range(K1):
                for mi in range(M1):
                    nc.tensor.matmul(
                        ps_l1[mi], w1[:, ki, mi * 128:(mi + 1) * 128],
                        x0[:, ki, :], start=(ki == 0), stop=(ki == K1 - 1))
            for mi in range(M1):
                nc.scalar.activation(out=h1[:, mi, :], in_=ps_l1[mi],
                                     func=Relu, bias=b1[:, mi:mi + 1])
            x_sb = x0
        else:
            cs = slice(c * NCHUNK, (c + 1) * NCHUNK)
            x_sb = acts.tile([128, K1, NCHUNK], MM_DT, tag="x")
            for ki in range(K1):
                nc.sync.dma_start(out=x_sb[:, ki, :], in_=xT_v[:, ki, cs])
            for mi in range(M1):
                ps = psum_mm.tile([128, NCHUNK], F32, tag="ps")
                msl = slice(mi * 128, (mi + 1) * 128)
                for ki in range(K1):
                    nc.tensor.matmul(ps, w1[:, ki, msl], x_sb[:, ki, :],
                                     start=(ki == 0), stop=(ki == K1 - 1))
                nc.scalar.activation(out=h1[:, mi, :], in_=ps, func=Relu,
                                     bias=b1[:, mi:mi + 1])
        emit_l4()

        # ---- layer 2: [500 -> 300(pad 384)], bias folded into W2 --------
        h2 = acts.tile([128, K3, NCHUNK], MM_DT, tag="h2")
        for mi in range(M2):
            ps = psum_mm.tile([128, NCHUNK], F32, tag="ps")
            msl = slice(mi * 128, (mi + 1) * 128)
            for ki in range(K2):
                nc.tensor.matmul(ps, w2[:, ki, msl], h1[:, ki, :],
                                 start=(ki == 0), stop=(ki == K2 - 1))
            nc.vector.tensor_scalar_max(h2[:, mi, :], ps, 0.0)

        # ---- layer 3: [300 -> 100(pad 128)], bias folded into W3 --------
        h3 = acts.tile([128, NCHUNK], MM_DT, tag="h3")
        ps = psum_mm.tile([128, NCHUNK], F32, tag="ps")
        for ki in range(K3):
            nc.tensor.matmul(ps, w3[:, ki, :], h2[:, ki, :],
                             start=(ki == 0), stop=(ki == K3 - 1))
        nc.scalar.activation(out=h3, in_=ps, func=Relu)

        pending_l4 = (c, h3)

    emit_l4()

    for pool in (psum_bm, psum_mm, scratch, outs, acts, consts):
        pool.release()


def _pst_store(nc, scratch, outs, bm, out_v, j):
    """PST epilogue on one batch-major [128, 8, 64] PSUM tile + store.

    `bm` holds v = x@W4 + b4 (bias folded into the matmul). possum is
    computed as (sum(t) + sum(|t|)) / 2; for all-negative rows both sums
    cancel exactly (identical reduction order) giving possum = 0, which the
    1e-38 floor turns into a huge-but-finite recip multiplied by
    max(t,0) = 0, reproducing the reference's denom=1 semantics.
    """
    G = BLOCKS_PER_BM
    Tanh = mybir.ActivationFunctionType.Tanh
    Sigm = mybir.ActivationFunctionType.Sigmoid
    Op = mybir.AluOpType

    o_sb = outs.tile([128, G, D_OUT], F32, tag="o")
    tb = scratch.tile([128, G, 63], F32, tag="tb")
    nc.scalar.activation(out=tb, in_=bm[:, :, 0:63], func=Tanh)
    nc.scalar.activation(out=o_sb[:, :, 63:64], in_=bm[:, :, 63:64], func=Sigm)

    s1 = scratch.tile([128, G], F32, tag="s1")
    nc.vector.reduce_sum(out=s1, in_=tb, axis=mybir.AxisListType.X)
    sa = scratch.tile([128, G], F32, tag="sa")
    nc.vector.tensor_reduce(out=sa, in_=tb, axis=mybir.AxisListType.X,
                            op=Op.add, apply_absolute_value=True)
    p = scratch.tile([128, G], F32, tag="p")
    nc.vector.tensor_tensor(out=p, in0=s1, in1=sa, op=Op.add)
    pg = scratch.tile([128, G], F32, tag="pg")
    nc.vector.tensor_scalar(out=pg, in0=p, scalar1=0.5, scalar2=1e-38,
                            op0=Op.mult, op1=Op.max)
    r = scratch.tile([128, G], F32, tag="r")
    nc.vector.reciprocal(r, pg)

    rb = r.unsqueeze(2).broadcast_to([128, G, 63])
    z = scratch.tile([128, G, 63], F32, tag="z")
    nc.vector.scalar_tensor_tensor(out=z, in0=tb, scalar=0.0, in1=rb,
                                   op0=Op.max, op1=Op.mult)
    nc.vector.scalar_tensor_tensor(out=o_sb[:, :, 0:63], in0=tb, scalar=0.0,
                                   in1=z, op0=Op.min, op1=Op.add)
    # per-block stores: each [128, 64] slice is a contiguous 32 KB DRAM
    # block, so these are cheap hardware queue triggers instead of one
    # ~640ns DIRECT2D descriptor-gen for the 3D scatter.
    for t in range(G):
        nc.sync.dma_start(out=out_v[j, :, t, :], in_=o_sb[:, t, :])


_PROG_CACHE = {}


def _build():
    if "nc" in _PROG_CACHE:
        return _PROG_CACHE["nc"]
    nc = bacc.Bacc("TRN2", target_bir_lowering=False, debug=False,
                   enable_asserts=False)
    aps = {
        "xT": nc.dram_tensor("xT", [D_IN, B], MM_DT, kind="ExternalInput").ap(),
        "W1": nc.dram_tensor("W1", [D_IN, 512], MM_DT, kind="ExternalInput").ap(),
        "b1": nc.dram_tensor("b1", [512], F32, kind="ExternalInput").ap(),
        "W2": nc.dram_tensor("W2", [512, 384], MM_DT, kind="ExternalInput").ap(),
        "W3": nc.dram_tensor("W3", [384, 128], MM_DT, kind="ExternalInput").ap(),
        "W4": nc.dram_tensor("W4", [128, D_OUT], MM_DT, kind="ExternalInput").ap(),
        "out": nc.dram_tensor("out", [B, D_OUT], F32, kind="ExternalOutput").ap(),
    }
    with tile.TileContext(nc) as tc:
        _emit(tc, aps)
    nc.compile()
    _PROG_CACHE["nc"] = nc
    return nc


def kernel(state, W1, b1, W2, b2, W3, b3, W4, b4, _trace=False):
    nc = _build()

    # Host-side padding: biases b2/b3/b4 are folded into a padded weight row,
    # with 1.0 markers chaining a constant-1 feature through the layers
    # (h1[500] via the b1 pad, h2[300] via W2[500,300], h3[100] via
    # W3[300,100]).
    W1p = np.zeros((512, 512), np.float32)
    W1p[:, :H1] = np.asarray(W1, np.float32)
    b1p = np.zeros((512,), np.float32)
    b1p[:H1] = np.asarray(b1, np.float32)
    b1p[H1] = 1.0
    W2p = np.zeros((512, 384), np.float32)
    W2p[:H1, :H2] = np.asarray(W2, np.float32)
    W2p[H1, :H2] = np.asarray(b2, np.float32)
    W2p[H1, H2] = 1.0
    W3p = np.zeros((384, 128), np.float32)
    W3p[:H2, :H3] = np.asarray(W3, np.float32)
    W3p[H2, :H3] = np.asarray(b3, np.float32)
    W3p[H2, H3] = 1.0
    W4p = np.zeros((128, D_OUT), np.float32)
    W4p[:H3, :] = np.asarray(W4, np.float32)
    W4p[H3, :] = np.asarray(b4, np.float32)

    weights = {
        "W1": W1p.astype(NP_MM), "b1": b1p, "W2": W2p.astype(NP_MM),
        "W3": W3p.astype(NP_MM), "W4": W4p.astype(NP_MM),
    }
    xT_all = np.asarray(state, np.float32).T.astype(NP_MM)  # [512, 65536]
    in_maps = []
    for i in range(N_CORES):
        in_maps.append(
            {"xT": np.ascontiguousarray(xT_all[:, i * B:(i + 1) * B]), **weights})

    res = run_bass_kernel_spmd(nc, in_maps, core_ids=list(range(N_CORES)),
                               trace=_trace)
    full = np.concatenate([res.results[i]["out"] for i in range(N_CORES)], axis=0)
    if _trace:
        kernel.last_results = res
    return full
